# revision 9
# baseline (speedup 1.0000x reference)
"""Trainium2 Bass kernel for nn_CartesianProductClassifier.

out[b,i,j] = sigmoid(MLP(concat(x[b,j], x[b,i])))  for x [8, 512, 32].

Math restructuring:
  layer1: h1[b,i,j] = relu(A[b,j] + C[b,i])   with A = x@W1_top (N-sized),
          C = x@W1_bot + b1 (N-sized)  -> first layer is O(N), not O(N^2).
  layer2: four concurrent 64x64 quadrant matmuls (tile_position) per twin;
  layer3: two col-grouped matmuls; layer4: column-shifted W4 accumulating
  all 32 twins of a batch-pair into one PSUM bank.

PE: each phase loads its stationary ONCE via an explicit ldweights and the
matmuls run with ldweights=False, so quad/pair matmuls execute
concurrently instead of serializing on per-matmul weight loads.

PSUM: one [128, 3, 1024] ring (6 banks, slot g%3).  Layer-2 output pq(g)
fills slot g; layer-3 output reuses bank 0 of the same slot (WAR on the
h2 eviction it depends on anyway).  Twin PAIRS are evicted with single
instructions via strided APs: Scalar relu FD=2048 for h2, DVE FD=2x512
for h3 -- amortizing per-instruction overhead.

Software pipeline, one wave per twin-pair k:
  h1(pair k+2) | L2+h2(pair k+1) | L3+h3(pair k) | L4(pair k-1)

Sharding: core c handles rows i in [64c, 64c+64) of all 8 batches.
"""

import numpy as np

B, N, D = 8, 512, 32
NCORES = 8
RPC = N // NCORES  # rows per core = 64

_PROG = None


def _build_program():
    import concourse.mybir as mybir
    import concourse.tile as tile
    from concourse import bacc

    dt = mybir.dt
    F32 = dt.float32
    BF16 = dt.bfloat16
    AF = mybir.ActivationFunctionType
    OP = mybir.AluOpType

    nc = bacc.Bacc(
        "TRN2", target_bir_lowering=False, debug=False, num_devices=NCORES
    )

    # xT2: [64, 4*512] col = bp*512 + j; rows 0:32 features of batch 2bp,
    #      rows 32:64 of batch 2bp+1.   xcT2: same layout, col = bp*64 + i.
    xT2 = nc.dram_tensor("xT2", [2 * D, 4 * N], BF16, kind="ExternalInput")
    # wcomb packs [xcT2 (256) | w1tbd (128) | w1bbd (128)] on 64 partitions
    wcomb = nc.dram_tensor("wcomb", [2 * D, 512], BF16, kind="ExternalInput")
    # w23 packs [w2full (128) | w3full (128)] on 128 partitions
    w23 = nc.dram_tensor("w23", [128, 256], BF16, kind="ExternalInput")
    w4sh = nc.dram_tensor("w4sh", [128, 32 * 128], BF16, kind="ExternalInput")
    # bcomb packs [b1s | b2s | b3s | b4s] columns
    bcomb = nc.dram_tensor("bcomb", [128, 4], F32, kind="ExternalInput")
    out = nc.dram_tensor("out", [B, RPC, N], F32, kind="ExternalOutput")

    with tile.TileContext(nc) as tc:
        with (
            tc.tile_pool(name="const", bufs=1) as const,
            tc.tile_pool(name="h1p", bufs=10) as h1p,
            tc.tile_pool(name="h2p", bufs=3) as h2p,
            tc.tile_pool(name="h3p", bufs=4) as h3p,
            tc.tile_pool(name="sigp", bufs=2) as sigp,
            tc.tile_pool(name="psR", bufs=1, space="PSUM") as psR,
            tc.tile_pool(name="psC", bufs=2, space="PSUM") as psC,
        ):
            # ---------- constant loads ----------
            xT2_sb = const.tile([2 * D, 4 * N], BF16, tag="xT2")
            nc.sync.dma_start(xT2_sb[:], xT2[:])
            wcomb_sb = const.tile([2 * D, 512], BF16, tag="wcomb")
            nc.sync.dma_start(wcomb_sb[:], wcomb[:])
            w23_sb = const.tile([128, 256], BF16, tag="w23")
            nc.sync.dma_start(w23_sb[:], w23[:])
            bcomb_sb = const.tile([128, 4], F32, tag="bcomb")
            nc.sync.dma_start(bcomb_sb[:], bcomb[:])
            xcT2_sb = wcomb_sb[:, 0:256]
            w1tbd_sb = wcomb_sb[:, 256:384]
            w1bbd_sb = wcomb_sb[:, 384:512]
            w2full_sb = w23_sb[:, 0:128]
            w3full_sb = w23_sb[:, 128:256]
            b1s_sb = bcomb_sb[:, 0:1]
            b2s_sb = bcomb_sb[:, 1:2]
            b3s_sb = bcomb_sb[:, 2:3]
            b4s_sb = bcomb_sb[:, 3:4]
            w4sh_sb = const.tile([128, 32 * 128], BF16, tag="w4sh")
            for q in range(4):
                # chunk q only gates mm4 twins 8q..8q+7
                nc.sync.dma_start(
                    w4sh_sb[:, q * 1024 : (q + 1) * 1024],
                    w4sh[:, q * 1024 : (q + 1) * 1024],
                )

            # PSUM ring: slot g%3; [:, s, 0:512] doubles as ps3 after h2.
            PS = psR.tile([128, 3, 1024], F32, tag="ring")

            # ---------- stage A: AT2 [128, 2048], Cpp [128, 256] ----------
            AT2 = const.tile([128, 4 * N], BF16, tag="AT2")
            Cpp = const.tile([128, 4 * RPC], F32, tag="Cpp")
            for bp in range(4):
                psa = PS[:, bp % 3, 0:N]
                nc.tensor.matmul(
                    psa,
                    w1tbd_sb,
                    xT2_sb[:, bp * N : (bp + 1) * N],
                    start=True,
                    stop=True,
                )
                # scalar copy keeps stage-A eviction off the vector engine
                nc.scalar.activation(
                    AT2[:, bp * N : (bp + 1) * N], psa, AF.Copy
                )
            psc = PS[:, 1, N : N + 4 * RPC]
            nc.tensor.matmul(
                psc,
                w1bbd_sb,
                xcT2_sb,
                start=True,
                stop=True,
            )
            # Cpp = psc + b1 (no relu here; relu happens after adding A)
            nc.vector.tensor_scalar(Cpp[:], psc, b1s_sb, None, OP.add)

            # ---------- main loop: 64 twin-pairs, software-pipelined ----
            # Twin g covers i-rows {2t, 2t+1} of batch-pair bp (g=32*bp+t).
            NT = 128
            h1_tiles = {}
            h2_tiles = {}
            h3_tiles = {}
            sig_tiles = {}

            def s_of(g):
                return g % 3

            def emit_h1(g):
                bp, t = g >> 5, g & 31
                at = AT2[:, bp * N : (bp + 1) * N]
                pair = []
                for half in range(2):
                    i = 2 * t + half
                    h1 = h1p.tile(
                        [128, N], BF16, tag="h1", name=f"h1_{g}_{half}"
                    )
                    nc.vector.tensor_scalar(
                        h1[:],
                        at,
                        Cpp[:, bp * RPC + i : bp * RPC + i + 1],
                        0.0,
                        OP.add,
                        OP.max,
                    )
                    pair.append(h1)
                h1_tiles[g] = pair

            def emit_l2(k):
                # one ldweights, then 8 non-self-loading quadrant matmuls
                # (4 per twin), then ONE FD=2048 relu over both slots.
                ga, gb = 2 * k, 2 * k + 1
                nc.tensor.ldweights(w2full_sb)
                for g in (ga, gb):
                    h1a, h1b = h1_tiles.pop(g)
                    pq = PS[:, s_of(g), :]
                    # bank0 = batch-even [z2(ia); z2(ib)], bank1 = batch-odd
                    for (rpos, cpos, h1t) in (
                        (0, 0, h1a),
                        (64, 0, h1a),
                        (0, 64, h1b),
                        (64, 64, h1b),
                    ):
                        mm = nc.tensor.matmul(
                            pq[cpos : cpos + 64, (0 if rpos == 0 else N) :
                               (N if rpos == 0 else 2 * N)],
                            w2full_sb[rpos : rpos + 64, cpos : cpos + 64],
                            h1t[rpos : rpos + 64, :],
                            start=True,
                            stop=True,
                            tile_position=(rpos, cpos),
                            skip_group_check=True,
                        )
                        mm.ins.ldweights = False
                sa, sb_ = s_of(ga), s_of(gb)
                lo, hi = min(sa, sb_), max(sa, sb_)
                pair_in = PS[:, lo : hi + 1 : (hi - lo), :]
                h2pq = h2p.tile([128, 2 * 2 * N], BF16, tag="h2", name=f"h2_{k}")
                nc.scalar.activation(
                    h2pq[:], pair_in, AF.Relu, bias=b2s_sb, scale=1.0
                )
                # column offset of twin g inside h2pq
                h2_tiles[k] = (h2pq, {ga: (0 if sa == lo else 1),
                                      gb: (0 if sb_ == lo else 1)})

            def emit_l3(k):
                ga, gb = 2 * k, 2 * k + 1
                h2pq, off = h2_tiles.pop(k)
                nc.tensor.ldweights(w3full_sb)
                for g in (ga, gb):
                    o = off[g] * 2 * N
                    ps3 = PS[:, s_of(g), 0:N]
                    # ps3[0:64] = batch-even [z3(ia); z3(ib)], [64:128] odd
                    for (cpos, sl) in ((0, slice(o, o + N)),
                                       (64, slice(o + N, o + 2 * N))):
                        mm = nc.tensor.matmul(
                            ps3[cpos : cpos + 64, :],
                            w3full_sb[:, cpos : cpos + 64],
                            h2pq[:, sl],
                            start=True,
                            stop=True,
                            tile_position=(0, cpos),
                            skip_group_check=True,
                        )
                        mm.ins.ldweights = False
                sa, sb_ = s_of(ga), s_of(gb)
                lo, hi = min(sa, sb_), max(sa, sb_)
                pair_in = PS[:, lo : hi + 1 : (hi - lo), 0:N]
                h3 = h3p.tile([128, 2 * N], BF16, tag="h3", name=f"h3_{k}")
                nc.vector.tensor_scalar(
                    h3[:], pair_in, b3s_sb, 0.0, OP.add, OP.max
                )
                h3_tiles[k] = (h3, {ga: (0 if sa == lo else 1),
                                    gb: (0 if sb_ == lo else 1)})

            def emit_l4(k):
                ga, gb = 2 * k, 2 * k + 1
                h3, off = h3_tiles.pop(k)
                for g in (ga, gb):
                    bp, t = g >> 5, g & 31
                    if t == 0:
                        sig_tiles[bp] = psC.tile(
                            [128, N], F32, tag="sig", name=f"sig{bp}"
                        )
                    o = off[g] * N
                    nc.tensor.matmul(
                        sig_tiles[bp][:],
                        w4sh_sb[:, t * 128 : (t + 1) * 128],
                        h3[:, o : o + N],
                        start=(t == 0),
                        stop=(t == 31),
                        skip_group_check=True,
                    )
                    if t == 31:
                        sig_sb = sigp.tile(
                            [128, N], F32, tag="sig_sb", name=f"sigsb{bp}"
                        )
                        nc.scalar.activation(
                            sig_sb[:],
                            sig_tiles.pop(bp)[:],
                            AF.Sigmoid,
                            bias=b4s_sb,
                            scale=1.0,
                        )
                        nc.sync.dma_start(out[2 * bp, :, :], sig_sb[0:64, :])
                        nc.sync.dma_start(
                            out[2 * bp + 1, :, :], sig_sb[64:128, :]
                        )

            NP = NT // 2  # 64 twin-pairs
            emit_h1(0)
            emit_h1(1)
            emit_h1(2)
            emit_h1(3)
            emit_l2(0)
            for k in range(NP + 1):
                if k + 2 < NP:
                    emit_h1(2 * k + 4)
                    emit_h1(2 * k + 5)
                if k + 1 < NP:
                    emit_l2(k + 1)
                if k < NP:
                    emit_l3(k)
                if k >= 1:
                    emit_l4(k - 1)

    nc.compile()
    return nc


def _get_program():
    global _PROG
    if _PROG is None:
        _PROG = _build_program()
    return _PROG


def prep_in_maps(inputs):
    import ml_dtypes

    x = np.ascontiguousarray(np.asarray(inputs["x"], dtype=np.float32))
    W1 = np.asarray(inputs["W1"], dtype=np.float32)
    b1 = np.asarray(inputs["b1"], dtype=np.float32)
    W2 = np.asarray(inputs["W2"], dtype=np.float32)
    b2 = np.asarray(inputs["b2"], dtype=np.float32)
    W3 = np.asarray(inputs["W3"], dtype=np.float32)
    b3 = np.asarray(inputs["b3"], dtype=np.float32)
    W4 = np.asarray(inputs["W4"], dtype=np.float32)
    b4 = np.asarray(inputs["b4"], dtype=np.float32)

    bf16 = ml_dtypes.bfloat16
    w1tbd = np.zeros((2 * D, 128), bf16)
    w1tbd[:D, :64] = W1[:D].astype(bf16)
    w1tbd[D:, 64:] = W1[:D].astype(bf16)
    w1bbd = np.zeros((2 * D, 128), bf16)
    w1bbd[:D, :64] = W1[D:].astype(bf16)
    w1bbd[D:, 64:] = W1[D:].astype(bf16)
    # w23: [w2full | w3full].
    # w2full = W2 in all four 64x64 quadrants (one full-array ldweights
    # feeds the four quadrant matmuls).
    # w3full = [w3bd | w3bd]: block-diag mapping [h2(i-even); h2(i-odd)]
    # feats -> [z3(i-even); z3(i-odd)], duplicated for both col-halves.
    w23 = np.zeros((128, 256), bf16)
    w2b = W2.astype(bf16)
    w23[:64, 0:64] = w2b
    w23[64:, 0:64] = w2b
    w23[:64, 64:128] = w2b
    w23[64:, 64:128] = w2b
    w3b = W3.astype(bf16)
    for half in range(2):
        c0 = 128 + 64 * half
        w23[:64, c0 : c0 + 32] = w3b
        w23[64:, c0 + 32 : c0 + 64] = w3b
    # w4sh[t]: h3 row-block r -> output column
    #   r=0 (b-even, ia) -> 2t      r=1 (b-even, ib) -> 2t+1
    #   r=2 (b-odd,  ia) -> 64+2t   r=3 (b-odd,  ib) -> 64+2t+1
    w4sh = np.zeros((128, 32 * 128), bf16)
    w4c = W4[:, 0].astype(bf16)
    for t in range(32):
        cols = (2 * t, 2 * t + 1, 64 + 2 * t, 64 + 2 * t + 1)
        for rr in range(4):
            w4sh[32 * rr : 32 * rr + 32, t * 128 + cols[rr]] = w4c

    # xT2 [64, 4*512]: col = bp*512 + j, rows 0:32 batch 2bp, 32:64 batch 2bp+1
    xT2 = np.zeros((2 * D, 4 * N), bf16)
    for bp in range(4):
        xT2[:D, bp * N : (bp + 1) * N] = x[2 * bp].T.astype(bf16)
        xT2[D:, bp * N : (bp + 1) * N] = x[2 * bp + 1].T.astype(bf16)

    bcomb = np.zeros((128, 4), np.float32)
    bcomb[:, 0] = np.concatenate([b1, b1])
    bcomb[:, 1] = np.concatenate([b2, b2])
    bcomb[:, 2] = np.tile(b3, 4)
    bcomb[:, 3] = b4[0]

    common = {
        "xT2": xT2,
        "w23": w23,
        "w4sh": w4sh,
        "bcomb": bcomb,
    }
    in_maps = []
    for c in range(NCORES):
        xc = x[:, c * RPC : (c + 1) * RPC, :]  # [8, 64, 32]
        wcomb = np.zeros((2 * D, 512), bf16)
        for bp in range(4):
            wcomb[:D, bp * RPC : (bp + 1) * RPC] = xc[2 * bp].T.astype(bf16)
            wcomb[D:, bp * RPC : (bp + 1) * RPC] = xc[2 * bp + 1].T.astype(bf16)
        wcomb[:, 256:384] = w1tbd
        wcomb[:, 384:512] = w1bbd
        in_maps.append({**common, "wcomb": wcomb})
    return in_maps


def assemble(per_core_outs):
    full = np.empty((B, N, N), np.float32)
    for c in range(NCORES):
        full[:, c * RPC : (c + 1) * RPC, :] = per_core_outs[c]
    return full


def run(inputs, trace=False):
    """Returns (full_output, BassKernelResults)."""
    from concourse.bass_utils import run_bass_kernel_spmd

    nc = _get_program()
    in_maps = prep_in_maps(inputs)
    res = run_bass_kernel_spmd(nc, in_maps, list(range(NCORES)), trace=trace)
    full = assemble([res.results[c]["out"] for c in range(NCORES)])
    return full, res


def kernel(**inputs):
    full, _ = run(inputs, trace=False)
    return full
